# revision 14
# baseline (speedup 1.0000x reference)
"""Banded multi-head attention (B=2, L=1024, D=1024, H=16, band W=64) on 8
Trainium2 NeuronCores.

Sharding: core = (batch b, head-group g) with 2 batches x 4 head groups of 4
heads each.  Each core computes q/k/v projections for its group (f32r
matmuls), the banded attention for its 4 heads, and a partial output
projection through its slice of Wo.  Host sums the 4 partial outputs per
batch.

Device layout notes:
- All matmul operands are pre-transposed on host so every DMA is contiguous:
  xT [din, L], wqT/wkT [din, dq] (lhsT), wvT [din, dv] (rhs), woT [dv, dout].
- Scores are computed transposed, S^T[span_key, query], in 3 chunks of 128
  keys per 256-query tile; the key axis is padded left by 128 (65 zeros + 63
  learned cache entries) so every chunk is a full 128 partitions.
- Band mask (+scale 1/sqrt(dh)) is applied with one scalar_tensor_tensor per
  head-pair chunk; exp on the scalar engine writes f32r attention weights.
- V is stored token-major with a ones-column per head; the attn @ V matmul
  then yields both o^T[dv, query] and the softmax denominator row.
- Normalization: fast reciprocal of the denominator row, broadcast across 64
  partitions with a K=1 fp32 matmul, one tensor-tensor multiply.
"""
import numpy as np

import concourse.bacc as bacc
import concourse.mybir as mybir
import concourse.tile as tile
from concourse import bass_utils

B, L, D, H, W = 2, 1024, 1024, 16, 64
DH = D // H           # 64
G = 4                 # head groups
HPG = H // G          # 4 heads per group
DG = D // G           # 256 dims per group
NCORES = 8

F32 = mybir.dt.float32
F32R = mybir.dt.float32r
NEG = -1.0e30


def _pin_exp_ln_table(arch: str):
    """Make Exp/Ln resolve only to the natural_log_exp_and_others act-func
    set so the table-load pass emits one load instead of alternating table
    swaps between the exp-only and ln-only sets (which wedges the device)."""
    import concourse.hw_specs as hw_specs
    tables = hw_specs.get_activation_tables(arch)   # cached, mutable
    exp = mybir.ActivationFunctionType.Exp
    ln = mybir.ActivationFunctionType.Ln
    assert "natural_log_exp_and_others" in tables
    for name, funcs in tables.items():
        if name != "natural_log_exp_and_others":
            funcs.discard(exp)
            funcs.discard(ln)


def build(repeat: int = 1, variant: str = "full"):
    """Build the per-core Bass program. Returns the compiled Bacc."""
    nc = bacc.Bacc("TRN2", target_bir_lowering=False, debug=False)
    _pin_exp_ln_table(nc.m.arch)

    xT = nc.dram_tensor("xT", [D, L], F32R, kind="ExternalInput")
    wqT = nc.dram_tensor("wqT", [D, DG], F32R, kind="ExternalInput")
    wkT = nc.dram_tensor("wkT", [D, DG], F32R, kind="ExternalInput")
    wvT = nc.dram_tensor("wvT", [D, DG], F32R, kind="ExternalInput")
    woT = nc.dram_tensor("woT", [DG, D], F32R, kind="ExternalInput")
    kc = nc.dram_tensor("kc", [DG, 128], F32R, kind="ExternalInput")
    vc = nc.dram_tensor("vc", [128, HPG * (DH + 1)], F32R, kind="ExternalInput")
    onesr = nc.dram_tensor("onesr", [128, 32], F32R, kind="ExternalInput")
    onesf = nc.dram_tensor("onesf", [1, 64], F32, kind="ExternalInput")
    maskd = nc.dram_tensor("mask", [3, 128, 512], F32, kind="ExternalInput")
    y = nc.dram_tensor("y", [L, D], F32, kind="ExternalOutput")

    VSLOT = DH + 1                    # 65 cols per (slot, head)
    VROW = HPG * VSLOT                # 260 cols per slot
    NSLOT = L // 128 + 1              # 9 slots (slot 0 = cache block)

    with tile.TileContext(nc) as tc:
        with tc.tile_pool(name="res", bufs=1) as res, \
             tc.tile_pool(name="epool", bufs=4) as epool, \
             tc.tile_pool(name="rcpool", bufs=2) as rcpool, \
             tc.tile_pool(name="ypool", bufs=3) as ypool, \
             tc.tile_pool(name="pj", bufs=3, space="PSUM") as pjp, \
             tc.tile_pool(name="stp", bufs=3, space="PSUM") as stp, \
             tc.tile_pool(name="op", bufs=2, space="PSUM") as opp:

            # ---- resident SBUF tensors -------------------------------------
            xk = [res.tile([128, L], F32R, tag=f"xk{k}", name=f"xk{k}") for k in range(8)]
            wqk = [res.tile([128, DG], F32R, tag=f"wq{k}", name=f"wq{k}") for k in range(8)]
            wkk = [res.tile([128, DG], F32R, tag=f"wk{k}", name=f"wk{k}") for k in range(8)]
            wvk = [res.tile([128, DG], F32R, tag=f"wv{k}", name=f"wv{k}") for k in range(8)]
            wo_sb = [res.tile([128, D], F32R, tag=f"wo{m}", name=f"wo{m}") for m in range(2)]
            qT = [res.tile([64, L], F32R, tag=f"qT{h}", name=f"qT{h}") for h in range(4)]
            kT = [res.tile([64, 128 + L], F32R, tag=f"kT{h}", name=f"kT{h}") for h in range(4)]
            v_sb = res.tile([128, NSLOT * VROW], F32R, tag="v", name="v_sb")
            mask_sb = res.tile([128, 3 * 512], F32, tag="mask", name="mask_sb")
            oT = [res.tile([128, L], F32R, tag=f"oT{m}", name=f"oT{m}") for m in range(2)]
            ones_sb = res.tile([1, 64], F32, tag="ones", name="ones_sb")

            for rep in range(repeat):
                # ---- init (constants shipped from host) ------------------
                ones_cols = v_sb[:, VROW:].rearrange(
                    "p (n c) -> p n c", c=VSLOT)[:, :, DH:DH + 1]
                nc.sync.dma_start(ones_cols, onesr.ap()[:, 0:32].unsqueeze(2))
                nc.sync.dma_start(ones_sb[:], onesf.ap())

                # ---- input DMAs ------------------------------------------
                for k in range(8):
                    nc.sync.dma_start(xk[k][:], xT.ap()[k * 128:(k + 1) * 128, :])
                for k in range(8):
                    nc.sync.dma_start(wqk[k][:], wqT.ap()[k * 128:(k + 1) * 128, :])
                    nc.sync.dma_start(wkk[k][:], wkT.ap()[k * 128:(k + 1) * 128, :])
                    nc.sync.dma_start(wvk[k][:], wvT.ap()[k * 128:(k + 1) * 128, :])
                for m in range(2):
                    nc.sync.dma_start(wo_sb[m][:], woT.ap()[m * 128:(m + 1) * 128, :])
                for h in range(4):
                    nc.sync.dma_start(kT[h][:, 0:128],
                                      kc.ap()[h * 64:(h + 1) * 64, :])
                nc.sync.dma_start(v_sb[:, 0:VROW], vc.ap())
                nc.sync.dma_start(
                    mask_sb[:].rearrange("p (s n) -> p s n", s=3),
                    maskd.ap().rearrange("s p n -> p s n"),
                )

                # ---- q/k projections (head-major: out [dq, tok]) ---------
                for wt, dst, off in ((wqk, qT, 0), (wkk, kT, 128)):
                    for m in range(2):
                        for n in range(2):
                            pt = pjp.tile([128, 512], F32, tag="pj", name="pj")
                            for k in range(8):
                                nc.tensor.matmul(
                                    pt[:],
                                    wt[k][:, m * 128:(m + 1) * 128],
                                    xk[k][:, n * 512:(n + 1) * 512],
                                    start=(k == 0), stop=(k == 7),
                                )
                            for hh in range(2):
                                nc.scalar.copy(
                                    dst[2 * m + hh][:, off + n * 512:
                                                    off + n * 512 + 512],
                                    pt[hh * 64:(hh + 1) * 64, :])

                # ---- v projection (token-major: out [tok, dv]) -----------
                for t in range(8):
                    pv = pjp.tile([128, DG], F32, tag="pj", name="pjv")
                    for k in range(8):
                        nc.tensor.matmul(
                            pv[:],
                            xk[k][:, t * 128:(t + 1) * 128],
                            wvk[k][:],
                            start=(k == 0), stop=(k == 7),
                        )
                    si = t + 1
                    dst = v_sb[:, si * VROW:(si + 1) * VROW].rearrange(
                        "p (h c) -> p h c", c=VSLOT)[:, :, 0:DH]
                    nc.vector.tensor_copy(
                        dst, pv[:].rearrange("p (h c) -> p h c", c=DH))

                # ---- banded attention ------------------------------------
                attn_tiles = 0 if variant == "proj" else 4
                for ti in range(attn_tiles):  # 256-query tiles
                    t0 = ti * 256
                    for m in range(2):       # head pairs
                        es = []
                        for s in range(3):   # 128-key span chunks
                            st = stp.tile([128, 512], F32, tag="st", name="st")
                            hh_range = [0] if variant == "scores0" else range(2)
                            for hh in hh_range:
                                h = 2 * m + hh
                                nc.tensor.matmul(
                                    st[:, hh * 256:(hh + 1) * 256],
                                    kT[h][:, t0 + s * 128: t0 + s * 128 + 128],
                                    qT[h][:, t0:t0 + 256],
                                    start=True, stop=True,
                                )
                            e = epool.tile([128, 512], F32R, tag="e", name="e")
                            if variant in ("scmm", "scores0"):
                                nc.scalar.copy(e[:], st[:])
                                es.append(e)
                                continue
                            nc.vector.scalar_tensor_tensor(
                                st[:], st[:], float(DH) ** -0.5,
                                mask_sb[:, s * 512:(s + 1) * 512],
                                mybir.AluOpType.mult, mybir.AluOpType.add,
                            )
                            if variant == "scstt":
                                nc.scalar.copy(e[:], st[:])
                                es.append(e)
                                continue
                            nc.scalar.activation(
                                e[:], st[:], mybir.ActivationFunctionType.Exp)
                            es.append(e)
                        if variant in ("scores", "scmm", "scstt", "scores0"):
                            continue
                        for hh in range(2):
                            h = 2 * m + hh
                            op = opp.tile([128, DG], F32, tag="o", name="o")
                            for s in range(3):
                                si = 2 * ti + s
                                nc.tensor.matmul(
                                    op[0:65, :],
                                    v_sb[:, si * VROW + h * VSLOT:
                                         si * VROW + h * VSLOT + VSLOT],
                                    es[s][:, hh * 256:(hh + 1) * 256],
                                    start=(s == 0), stop=(s == 2),
                                )
                            oT_dst = oT[m][hh * 64:(hh + 1) * 64,
                                           t0:t0 + 256]
                            if variant == "nonorm":
                                nc.scalar.copy(oT_dst, op[0:64, :])
                                continue
                            rc = rcpool.tile([1, 256], F32, tag="rc", name="rc")
                            if variant == "norecip":
                                nc.vector.tensor_copy(rc[:], op[64:65, :])
                            else:
                                # ln(denom); 1/d applied as exp(-ln d) below
                                nc.scalar.activation(
                                    rc[:], op[64:65, :],
                                    mybir.ActivationFunctionType.Ln)
                            nc.tensor.matmul(op[64:128, :], ones_sb[:], rc[:],
                                             start=True, stop=True)
                            bc = rcpool.tile([64, 256], F32, tag="bc",
                                             name="bc")
                            if variant == "norecip":
                                nc.scalar.copy(bc[:], op[64:128, :])
                            else:
                                nc.scalar.activation(
                                    bc[:], op[64:128, :],
                                    mybir.ActivationFunctionType.Exp,
                                    scale=-1.0)
                            nc.vector.tensor_mul(oT_dst, op[0:64, :], bc[:])

                # ---- output projection -----------------------------------
                for t in range(8):
                    for n2 in range(2):
                        yp = pjp.tile([128, 512], F32, tag="pj", name="pj")
                        for m in range(2):
                            osrc = xk if variant in ("proj", "scores", "scmm", "scstt", "scores0") else oT
                            nc.tensor.matmul(
                                yp[:],
                                osrc[m][:, t * 128:(t + 1) * 128],
                                wo_sb[m][:, n2 * 512:(n2 + 1) * 512],
                                start=(m == 0), stop=(m == 1),
                            )
                        ysb = ypool.tile([128, 512], F32, tag="y", name="ysb")
                        if t % 2 == 0:
                            nc.scalar.copy(ysb[:], yp[:])
                        else:
                            nc.vector.tensor_copy(ysb[:], yp[:])
                        nc.sync.dma_start(
                            y.ap()[t * 128:(t + 1) * 128,
                                   n2 * 512:(n2 + 1) * 512],
                            ysb[:])

    nc.compile()
    return nc


def make_mask() -> np.ndarray:
    """[3, 128, 512] additive mask (0 in band, NEG outside), doubled for the
    two heads sharing one 512-wide score tile.  Chunk s, row r (key index
    t0 + s*128 + r - 128), query col i valid iff the key is within the
    64-wide causal band of query t0+i."""
    m = np.full((3, 128, 256), NEG, dtype=np.float32)
    for s in range(3):
        for r in range(128):
            lo = s * 128 + r - 128
            hi = s * 128 + r - 65
            lo_c = max(lo, 0)
            hi_c = min(hi, 255)
            if lo_c <= hi_c:
                m[s, r, lo_c:hi_c + 1] = 0.0
    return np.concatenate([m, m], axis=2)


def prep_inputs(x, Wq, Wk, Wv, Wo, last_k_init, last_v_init):
    """Shard + pre-transpose full inputs into 8 per-core input maps."""
    mask = make_mask()
    in_maps = []
    for core in range(NCORES):
        b, g = divmod(core, G)
        sl = slice(g * DG, (g + 1) * DG)
        lk = last_k_init[:, g * HPG:(g + 1) * HPG, :]   # [63, 4, 64]
        lv = last_v_init[:, g * HPG:(g + 1) * HPG, :]
        kcg = np.zeros((DG, 128), dtype=np.float32)
        kcg[:, 65:128] = lk.reshape(W - 1, DG).T
        vc = np.zeros((128, HPG * (DH + 1)), dtype=np.float32)
        for h in range(HPG):
            vc[65:128, h * (DH + 1):h * (DH + 1) + DH] = lv[:, h, :]
            vc[65:128, h * (DH + 1) + DH] = 1.0
        in_maps.append({
            "xT": np.ascontiguousarray(x[b].T),
            "wqT": np.ascontiguousarray(Wq[sl, :].T),
            "wkT": np.ascontiguousarray(Wk[sl, :].T),
            "wvT": np.ascontiguousarray(Wv[sl, :].T),
            "woT": np.ascontiguousarray(Wo[:, sl].T),
            "kc": kcg,
            "vc": vc,
            "onesr": np.ones((128, 32), dtype=np.float32),
            "onesf": np.ones((1, 64), dtype=np.float32),
            "mask": mask,
        })
    return in_maps


_built = None


def kernel(x, Wq, Wk, Wv, Wo, last_k_init, last_v_init) -> np.ndarray:
    global _built
    x = np.asarray(x, dtype=np.float32)
    args = [np.asarray(a, dtype=np.float32)
            for a in (Wq, Wk, Wv, Wo, last_k_init, last_v_init)]
    in_maps = prep_inputs(x, *args)
    if _built is None:
        _built = build()
    r = bass_utils.run_bass_kernel_spmd(
        _built, in_maps, core_ids=list(range(NCORES)))
    out = np.zeros((B, L, D), dtype=np.float32)
    for core in range(NCORES):
        b = core // G
        out[b] += r.results[core]["y"]
    return out
